# revision 25
# baseline (speedup 1.0000x reference)
"""Trainium2 Bass kernel for nn_Decoder (dense transformer decoder layer).

Strategy: pure data-parallel over batch B=256 across 8 NeuronCores (32
batches/core).  Each core runs the full decoder layer on its shard; no
collectives.

v2 design (vs baseline):
  - residual stream bf16 token-major; idx/mem shipped bf16 (halves input DMA)
  - LayerNorm stats via DVE bn_stats/bn_aggr; rsqrt = Exp(-0.5*Ln(var+eps))
    so the ONLY ACT table set used kernel-wide is natural_log_exp_and_others
    (exp/ln/identity/copy/relu) -> no table switching
  - memory's feature-major transpose done by DMA-transpose engines (bf16)
  - softmax denominator fused into the AV matmul via a per-head ones column
    appended to V (psum row 73 = Z); 1/Z via reciprocal_approx_fast
  - FFN bias folded into w1 as an extra K row against a ones-row in h3f;
    ReLU fused at the PSUM evict on ACT
  - per-head q/k slots evicted two heads at a time (one PSUM bank each)
  - samples processed two-at-a-time with stage-interleaved program order so
    the PE never sits behind a LayerNorm (HAM stays warm)
"""

import sys

sys.path.insert(0, "/opt/trn_rl_repo")

from contextlib import ExitStack

import numpy as np
import ml_dtypes

import concourse.bass as bass
import concourse.bacc as bacc
import concourse.mybir as mybir
import concourse.tile as tile
from concourse.bass_utils import run_bass_kernel_spmd

F32 = mybir.dt.float32
BF16 = mybir.dt.bfloat16
BF16NP = ml_dtypes.bfloat16
AF = mybir.ActivationFunctionType

B, T, E, H = 256, 200, 584, 8
HS = E // H  # 73
FF = 4 * E  # 2336
NCORES = 8
BL = B // NCORES  # 32
SCALE = float(E) ** -0.5
EPS = 1e-5
TP = 208  # mem DRAM padded token dim (multiple of 16 for DMA transpose)
EP = 640  # mem DRAM padded feature dim (multiple of 128)

# tile decompositions
TS = [(0, 128), (1, 72)]  # token tiles (T=200)
EB = [128, 128, 128, 128, 72]  # E=584 partition blocks
EK = 5
FFB = [128] * 18 + [32]  # FF=2336 partition blocks
FFK = 19
NSPL = [0, 292]  # free-dim split of an E-sized matmul output (<=512 psum)
HSZ = HS  # v slot width per head


def build_nc(bl=BL):
    """Build the single-core Bass program processing `bl` batch samples."""
    nc = bacc.Bacc(None, target_bir_lowering=False, debug=False)

    idx_d = nc.dram_tensor("idx", [bl, T, E], BF16, kind="ExternalInput")
    mem_d = nc.dram_tensor("mem", [bl, TP, EP], BF16, kind="ExternalInput")
    w_names = ["wq_sa", "wk_sa", "wv_sa", "wq_ca", "wk_ca", "wv_ca"]
    w_d = {n: nc.dram_tensor(n, [128, EK, E], BF16, kind="ExternalInput") for n in w_names}
    wp_sa_d = nc.dram_tensor("wp_sa", [128, H, E], BF16, kind="ExternalInput")
    wp_ca_d = nc.dram_tensor("wp_ca", [128, H, E], BF16, kind="ExternalInput")
    w1_d = nc.dram_tensor("w1", [128, EK, FF], BF16, kind="ExternalInput")
    w2_d = nc.dram_tensor("w2", [128, FFK, E], BF16, kind="ExternalInput")
    ident_d = nc.dram_tensor("ident", [128, 128], BF16, kind="ExternalInput")
    mask_d = nc.dram_tensor("mask", [128, 128], BF16, kind="ExternalInput")
    out_d = nc.dram_tensor("out", [bl, T, E], F32, kind="ExternalOutput")

    with tile.TileContext(nc) as tc, ExitStack() as ctx:
        wpool = ctx.enter_context(tc.tile_pool(name="wpool", bufs=1))
        w_sb = {}
        for n in w_names:
            w_sb[n] = wpool.tile([128, EK, E], BF16, name=n + "_sb")
            nc.sync.dma_start(w_sb[n][:], w_d[n][:])
        wp_sa_sb = wpool.tile([128, H, E], BF16, name="wp_sa_sb")
        nc.sync.dma_start(wp_sa_sb[:], wp_sa_d[:])
        wp_ca_sb = wpool.tile([128, H, E], BF16, name="wp_ca_sb")
        nc.sync.dma_start(wp_ca_sb[:], wp_ca_d[:])
        w1_sb = wpool.tile([128, EK, FF], BF16, name="w1_sb")
        nc.sync.dma_start(w1_sb[:], w1_d[:])
        w2_sb = wpool.tile([128, FFK, E], BF16, name="w2_sb")
        nc.sync.dma_start(w2_sb[:], w2_d[:])
        ident_sb = wpool.tile([128, 128], BF16, name="ident_sb")
        nc.sync.dma_start(ident_sb[:], ident_d[:])
        mask_sb = wpool.tile([128, 128], BF16, name="mask_sb")
        nc.sync.dma_start(mask_sb[:], mask_d[:])
        I32 = mybir.dt.int32
        shift1_sb = wpool.tile([128, 1], I32, name="shift1_sb")
        nc.vector.memset(shift1_sb[:], 1)
        ones_i_sb = wpool.tile([128, 1], I32, name="ones_i_sb")
        nc.vector.memset(ones_i_sb[:], -1)  # 0xFFFFFFFF for bitwise-not via xor
        magic_sb = wpool.tile([128, 2], I32, name="magic_sb")
        nc.vector.memset(magic_sb[:], 0x5F3759DF + 1)  # M+1 (M - x = ~x + M+1)
        ones_sb = wpool.tile([128, 1], BF16, name="ones_sb")
        nc.vector.memset(ones_sb[:], 1.0)

        resid = ctx.enter_context(tc.tile_pool(name="resid", bufs=2))
        work = ctx.enter_context(tc.tile_pool(name="work", bufs=2))
        stat = ctx.enter_context(tc.tile_pool(name="stat", bufs=4))
        opool = ctx.enter_context(tc.tile_pool(name="opool", bufs=2))
        ps_tp = ctx.enter_context(tc.tile_pool(name="ps_tp", bufs=1, space="PSUM"))
        ps_hp = ctx.enter_context(tc.tile_pool(name="ps_hp", bufs=2, space="PSUM"))
        ps_mm = ctx.enter_context(tc.tile_pool(name="ps_mm", bufs=2, space="PSUM"))
        ps_s = ctx.enter_context(tc.tile_pool(name="ps_s", bufs=2, space="PSUM"))
        ps_z = ctx.enter_context(tc.tile_pool(name="ps_z", bufs=1, space="PSUM"))

        def load(b):
            x1 = resid.tile([128, 2, E], BF16, name=f"x1_{b}", tag="x1", bufs=3)
            nc.sync.dma_start(x1[:, 0, :], idx_d[b, 0:128, :])
            nc.sync.dma_start(x1[0:72, 1, :], idx_d[b, 128:200, :])
            memf = work.tile([128, EK, TP], BF16, name=f"memf_{b}", tag="memf")
            for eb in range(EK):
                nc.sync.dma_start_transpose(
                    memf[:, eb, :], mem_d[b, :, eb * 128:eb * 128 + 128])
            return x1, memf

        def layernorm(x_t, name):
            """x_t [128,2,E] bf16 -> h_tok [128,2,E] bf16 normalized (no w/b).
            rsqrt via Newton iteration on DVE (quake seed + 2 NR passes) so no
            Sqrt/Ln activations are needed (single ACT table set kernel-wide)."""
            h_tok = work.tile([128, 2, E + 1], BF16, name=name, tag="htok", bufs=3)
            mv = stat.tile([128, 2, 2], F32, name=name + "_mv", tag="mv")
            for tt, tsz in TS:
                xs = x_t[0:tsz, tt, :]
                st = stat.tile([128, 2, 6], F32, name=name + f"_st_{tt}", tag="st")
                nc.vector.bn_stats(st[0:tsz, 0], xs[:, 0:292])
                nc.vector.bn_stats(st[0:tsz, 1], xs[:, 292:584])
                nc.vector.bn_aggr(mv[0:tsz, tt], st[0:tsz])
            AL = mybir.AluOpType
            vpe = stat.tile([128, 2], F32, name=name + "_vp", tag="vp")
            nc.vector.tensor_scalar(vpe[:], mv[:, :, 1], EPS, None, AL.add)
            r = stat.tile([128, 2], F32, name=name + "_r", tag="r")
            # seed bits = M - (v_bits >> 1) = ((v_bits >> 1) ^ ~0) + (M+1)
            nc.vector.tensor_scalar(
                r.bitcast(mybir.dt.int32)[:], vpe.bitcast(mybir.dt.int32)[:],
                shift1_sb[:], ones_i_sb[:], AL.arith_shift_right, AL.bitwise_xor)
            nc.vector.tensor_tensor(
                r.bitcast(mybir.dt.int32)[:], r.bitcast(mybir.dt.int32)[:],
                magic_sb[:], AL.add)
            for _ in range(1):  # r *= 1.5 - 0.5*v*r*r
                t = stat.tile([128, 2], F32, name=name + "_t", tag="t")
                nc.vector.tensor_mul(t[:], r[:], r[:])
                nc.vector.tensor_mul(t[:], t[:], vpe[:])
                nc.vector.tensor_scalar(t[:], t[:], -0.5, 1.5, AL.mult, AL.add)
                nc.vector.tensor_mul(r[:], r[:], t[:])
            nmr = stat.tile([128, 2], F32, name=name + "_nm", tag="nm")
            nc.vector.tensor_tensor(nmr[:], mv[:, :, 0], r[:], AL.mult)
            nc.vector.tensor_scalar(nmr[:], nmr[:], -1.0, None, AL.mult)
            for tt, tsz in TS:
                nc.scalar.activation(
                    h_tok[0:tsz, tt, 0:E], x_t[0:tsz, tt, :], AF.Identity,
                    bias=nmr[0:tsz, tt:tt + 1], scale=r[0:tsz, tt:tt + 1])
            return h_tok

        def to_fm(src_tok, name, ones_row=False):
            """[128,2,E(+1)] bf16 token-major -> [128,EK,T] bf16 feature-major.
            ones_row: src col E is set to 1.0 and carried through the block-4
            transpose into feature row 72 (the FFN bias row)."""
            dst = work.tile([128, EK, T], BF16, name=name, tag="hfm", bufs=3)
            e4 = 73 if ones_row else 72
            if ones_row:
                nc.vector.memset(src_tok[:, :, E:E + 1], 1.0)
            for tt, tsz in TS:
                ps = ps_tp.tile([128, EK, 128], BF16, name=f"{name}_tp{tt}", tag="tp")
                for eb in range(4):
                    nc.tensor.transpose(
                        ps[0:128, eb, 0:tsz],
                        src_tok[0:tsz, tt, eb * 128:eb * 128 + 128],
                        ident_sb[0:tsz, 0:tsz])
                nc.tensor.transpose(
                    ps[0:e4, 4, 0:tsz],
                    src_tok[0:tsz, tt, 512:512 + e4],
                    ident_sb[0:tsz, 0:tsz])
                nc.vector.tensor_copy(
                    dst[:, 0:4, tt * 128:tt * 128 + tsz], ps[:, 0:4, 0:tsz])
                nc.vector.tensor_copy(
                    dst[0:e4, 4, tt * 128:tt * 128 + tsz], ps[0:e4, 4, 0:tsz])
            return dst

        def proj_qk(w, src_fm, name):
            """q/k projection -> per-head aligned [HS, H, T] bf16."""
            dst = work.tile([HS, H, T], BF16, name=name, tag=name[:1])
            for hp in range(4):
                ps = ps_hp.tile([HS, 2, T], F32, name=f"{name}_ps{hp}", tag="hp")
                for h2 in range(2):
                    h = 2 * hp + h2
                    for k in range(EK):
                        nc.tensor.matmul(
                            ps[:, h2, :], w[0:EB[k], k, HS * h:HS * h + HS],
                            src_fm[0:EB[k], k, 0:T],
                            start=(k == 0), stop=(k == EK - 1))
                nc.scalar.copy(dst[:, 2 * hp:2 * hp + 2, :], ps[:])
            return dst

        def proj_v(w, src_fm, name):
            """v projection token-major with per-head ones column:
            [128, 2, H, HSZ] bf16 (col HS of each head slot = 1.0)."""
            dst = work.tile([128, 2, H, HSZ], BF16, name=name, tag="vtok")
            for mt, msz in TS:
                for ni, n0 in enumerate(NSPL):
                    ps = ps_mm.tile([128, 292], F32, name=f"{name}_ps{mt}{ni}", tag="mm")
                    for k in range(EK):
                        nc.tensor.matmul(
                            ps[0:msz, :],
                            src_fm[0:EB[k], k, mt * 128:mt * 128 + msz],
                            w[0:EB[k], k, n0:n0 + 292],
                            start=(k == 0), stop=(k == EK - 1))
                    nc.vector.tensor_copy(
                        dst[0:msz, mt, 4 * ni:4 * ni + 4, 0:HS],
                        ps[0:msz, :].rearrange("p (h d) -> p h d", h=4))
            return dst

        def attn_scores(q, k, causal, name):
            """scores + exp for all heads -> expS [128, 2, H, T] bf16.
            One exp per head covers both s-tiles (unwritten psum regions get
            exp'd into never-read expS slots; harmless)."""
            expS = opool.tile([128, 2, H, T], BF16, name=name, tag="expS")
            for h in range(H):
                ps = ps_s.tile([128, 2, T], F32, name=f"{name}_s{h}", tag="s")
                nc.tensor.matmul(
                    ps[0:128, 0, :], k[:, h, 0:128], q[:, h, :],
                    start=True, stop=True)
                t0 = 128 if causal else 0
                nc.tensor.matmul(
                    ps[0:72, 1, t0:T], k[:, h, 128:200], q[:, h, t0:T],
                    start=True, stop=True)
                nc.scalar.activation(
                    expS[0:128, 0, h, :], ps[0:128, 0, :], AF.Exp, scale=SCALE)
                nc.scalar.activation(
                    expS[0:72, 1, h, t0:T], ps[0:72, 1, t0:T], AF.Exp, scale=SCALE)
                if causal:
                    nc.gpsimd.tensor_mul(
                        expS[0:128, 0, h, 0:128], expS[0:128, 0, h, 0:128],
                        mask_sb[0:128, 0:128])
                    nc.gpsimd.tensor_mul(
                        expS[0:72, 1, h, 128:200], expS[0:72, 1, h, 128:200],
                        mask_sb[0:72, 0:72])
            return expS

        def attn_av(expS, v, causal, name):
            """AV with fused Z row -> normalized o [HS, H, T] bf16."""
            osb = opool.tile([HS, H, T], BF16, name=name, tag="osb")
            for hp in range(4):
                ps = ps_hp.tile([HSZ, 2, T], F32, name=f"{name}_o{hp}", tag="hp")
                for h2 in range(2):
                    h = 2 * hp + h2
                    if causal:
                        nc.tensor.matmul(
                            ps[:, h2, 0:128], v[0:128, 0, h, :],
                            expS[0:128, 0, h, 0:128], start=True, stop=True)
                        nc.tensor.matmul(
                            ps[:, h2, 128:200], v[0:128, 0, h, :],
                            expS[0:128, 0, h, 128:200], start=True, stop=False)
                        nc.tensor.matmul(
                            ps[:, h2, 128:200], v[0:72, 1, h, :],
                            expS[0:72, 1, h, 128:200], start=False, stop=True)
                    else:
                        nc.tensor.matmul(
                            ps[:, h2, :], v[0:128, 0, h, :],
                            expS[0:128, 0, h, :], start=True, stop=False)
                        nc.tensor.matmul(
                            ps[:, h2, :], v[0:72, 1, h, :],
                            expS[0:72, 1, h, :], start=False, stop=True)
                h0 = 2 * hp
                zps = ps_z.tile([1, 2, T], F32, name=f"{name}_z{hp}", tag="z")
                if causal:
                    nc.tensor.matmul(zps[0:1, :, :], ones_sb[0:128, 0:1],
                                     expS[0:128, 0, h0:h0 + 2, :],
                                     start=True, stop=False)
                    nc.tensor.matmul(zps[0:1, :, 128:200], ones_sb[0:72, 0:1],
                                     expS[0:72, 1, h0:h0 + 2, 128:200],
                                     start=False, stop=True)
                else:
                    nc.tensor.matmul(zps[0:1, :, :], ones_sb[0:128, 0:1],
                                     expS[0:128, 0, h0:h0 + 2, :],
                                     start=True, stop=False)
                    nc.tensor.matmul(zps[0:1, :, :], ones_sb[0:72, 0:1],
                                     expS[0:72, 1, h0:h0 + 2, :],
                                     start=False, stop=True)
                zr = stat.tile([1, 2, T], F32, name=f"{name}_zr{hp}", tag="zr")
                nc.vector.reciprocal_approx_fast(out=zr[0:1, :, :], in_=zps[0:1, :, :])
                for h2 in range(2):
                    h = h0 + h2
                    zb = stat.tile([HS, T], F32, name=f"{name}_zb{h}", tag="zb")
                    nc.gpsimd.partition_broadcast(zb[:, :], zr[0:1, h2, :])
                    nc.vector.tensor_mul(osb[:, h, :], ps[0:HS, h2, :], zb[:, :])
            return osb

        def attn_proj(osb, wp_sb, x_in, name, xtag, xbufs=2):
            """projection (accumulate over heads) + residual, token-major."""
            x_out = resid.tile([128, 2, E], BF16, name=f"{name}_xo", tag=xtag,
                               bufs=xbufs)
            for mt, msz in TS:
                for n0 in NSPL:
                    ps = ps_mm.tile([128, 292], F32, name=f"{name}_pj", tag="mm")
                    for h in range(H):
                        nc.tensor.matmul(
                            ps[0:msz, :],
                            osb[:, h, mt * 128:mt * 128 + msz],
                            wp_sb[0:HS, h, n0:n0 + 292],
                            start=(h == 0), stop=(h == H - 1))
                    nc.vector.tensor_add(
                        x_out[0:msz, mt, n0:n0 + 292], ps[0:msz, :],
                        x_in[0:msz, mt, n0:n0 + 292])
            return x_out

        def ffn_w1(h3f, b):
            ff = work.tile([128, FFK, T], BF16, name=f"ff_{b}", tag="ff")
            for m in range(FFK):
                msz = FFB[m]
                ps = ps_mm.tile([128, 292], F32, name=f"f1_{b}_{m}", tag="mm")
                for k in range(EK):
                    ksz = 73 if k == EK - 1 else EB[k]  # incl. bias ones-row
                    nc.tensor.matmul(
                        ps[0:msz, 0:T], w1_sb[0:ksz, k, m * 128:m * 128 + msz],
                        h3f[0:ksz, k, :], start=(k == 0), stop=(k == EK - 1))
                nc.vector.tensor_scalar(
                    ff[0:msz, m, :], ps[0:msz, 0:T], 0.0, None,
                    mybir.AluOpType.max)
            return ff

        def ffn_w2(ff, x3, b):
            xo = resid.tile([128, 2, E], F32, name=f"xo_{b}", tag="xo")
            for mt, msz in TS:
                for n0 in NSPL:
                    ps = ps_mm.tile([128, 292], F32, name=f"f2_{b}_{mt}_{n0}", tag="mm")
                    for k in range(FFK):
                        nc.tensor.matmul(
                            ps[0:msz, :], ff[0:FFB[k], k, mt * 128:mt * 128 + msz],
                            w2_sb[0:FFB[k], k, n0:n0 + 292],
                            start=(k == 0), stop=(k == FFK - 1))
                    nc.vector.tensor_add(
                        xo[0:msz, mt, n0:n0 + 292], ps[0:msz, :],
                        x3[0:msz, mt, n0:n0 + 292])
            return xo

        def store(xo, b):
            nc.sync.dma_start(out_d[b, 0:128, :], xo[:, 0, :])
            nc.sync.dma_start(out_d[b, 128:200, :], xo[0:72, 1, :])

        def stages(b):
            """Generator of per-sample stages; yields after each stage so two
            samples can be interleaved in program order."""
            x1, memf = load(b)
            yield
            h1 = layernorm(x1, f"ln1_{b}")
            yield
            h1f = to_fm(h1, f"h1f_{b}")
            yield
            q1 = proj_qk(w_sb["wq_sa"], h1f, f"q1_{b}")
            k1 = proj_qk(w_sb["wk_sa"], h1f, f"k1_{b}")
            v1 = proj_v(w_sb["wv_sa"], h1f, f"v1_{b}")
            yield
            e1 = attn_scores(q1, k1, True, f"se1_{b}")
            yield
            o1 = attn_av(e1, v1, True, f"av1_{b}")
            yield
            x2 = attn_proj(o1, wp_sa_sb, x1, f"sa_{b}", "x2")
            yield
            h2 = layernorm(x2, f"ln2_{b}")
            yield
            h2f = to_fm(h2, f"h2f_{b}")
            yield
            q2 = proj_qk(w_sb["wq_ca"], h2f, f"q2_{b}")
            k2 = proj_qk(w_sb["wk_ca"], memf, f"k2_{b}")
            v2 = proj_v(w_sb["wv_ca"], h2f, f"v2_{b}")
            yield
            e2 = attn_scores(q2, k2, False, f"se2_{b}")
            yield
            o2 = attn_av(e2, v2, False, f"av2_{b}")
            yield
            x3 = attn_proj(o2, wp_ca_sb, x2, f"ca_{b}", "x3")
            yield
            h3 = layernorm(x3, f"ln3_{b}")
            yield
            h3f = to_fm(h3, f"h3f_{b}", ones_row=True)
            yield
            ff = ffn_w1(h3f, b)
            yield
            xo = ffn_w2(ff, x3, b)
            yield
            store(xo, b)

        # software-pipeline the samples: each sample starts HALF stages after
        # the previous one, so LayerNorm/softmax phases of one sample overlap
        # the matmul phases of its neighbor and the PE never idles (HAM warm).
        HALF = 9
        active = []
        t = 0
        next_s = 0
        while next_s < bl or active:
            if next_s < bl and t >= next_s * HALF:
                active.append(stages(next_s))
                next_s += 1
            for g in list(active):
                if next(g, "end") == "end":
                    active.remove(g)
            t += 1

    nc.compile()
    return nc


def _pack_kxm(w, nk, extra_row=None):
    """[K, M] fp32 -> [128, nk, M] bf16 with K zero-padded to 128*nk.
    extra_row, if given, is placed at global row K (the first pad row)."""
    K, M = w.shape
    pad = np.zeros((128 * nk, M), np.float32)
    pad[:K] = w
    if extra_row is not None:
        pad[K] = extra_row
    return np.ascontiguousarray(
        pad.reshape(nk, 128, M).transpose(1, 0, 2)).astype(BF16NP)


def prepare_inputs(inputs):
    """Host-side prep: LN folding, weight packing, per-core sharding."""
    f = {k: np.asarray(v, np.float32) for k, v in inputs.items()}

    def fold(lnw, lnb, w3):
        wf = w3 * lnw[None, :, None]
        bias = np.einsum("e,hed->hd", lnb, w3) if lnb.any() else 0.0
        assert np.allclose(bias, 0.0, atol=1e-12), "nonzero folded qkv bias unsupported"
        return wf

    sa_q = fold(f["ln1_w"], f["ln1_b"], f["sa_q"])
    sa_k = fold(f["ln1_w"], f["ln1_b"], f["sa_k"])
    sa_v = fold(f["ln1_w"], f["ln1_b"], f["sa_v"])
    ca_q = fold(f["ln2_w"], f["ln2_b"], f["ca_q"])
    ca_v = fold(f["ln2_w"], f["ln2_b"], f["ca_v"])
    ca_k = f["ca_k"]  # cross-attn keys come from raw memory (no LN)
    w1 = f["ff_w1"] * f["ln3_w"][:, None]
    b1 = f["ff_b1"] + f["ln3_b"] @ f["ff_w1"]
    assert np.allclose(f["sa_pb"], 0.0) and np.allclose(f["ca_pb"], 0.0), \
        "nonzero attn proj bias unsupported"
    assert np.allclose(f["ff_b2"], 0.0), "nonzero ff_b2 unsupported"

    def stack_heads(w3):  # [H, E, HS] -> [E, H*HS]
        return np.ascontiguousarray(w3.transpose(1, 0, 2)).reshape(E, E)

    def pack_proj(pw):  # [E, E] -> [128(73 used), H, E] per-head K layout
        r = pw.reshape(H, HS, E)
        out = np.zeros((H, 128, E), np.float32)
        out[:, :HS, :] = r
        return np.ascontiguousarray(out.transpose(1, 0, 2)).astype(BF16NP)

    shared = {
        "wq_sa": _pack_kxm(stack_heads(sa_q), EK),
        "wk_sa": _pack_kxm(stack_heads(sa_k), EK),
        "wv_sa": _pack_kxm(stack_heads(sa_v), EK),
        "wq_ca": _pack_kxm(stack_heads(ca_q), EK),
        "wk_ca": _pack_kxm(stack_heads(ca_k), EK),
        "wv_ca": _pack_kxm(stack_heads(ca_v), EK),
        "wp_sa": pack_proj(f["sa_pw"]),
        "wp_ca": pack_proj(f["ca_pw"]),
        "w1": _pack_kxm(w1, EK, extra_row=b1),
        "w2": _pack_kxm(f["ff_w2"], FFK),
        "ident": np.eye(128, dtype=BF16NP),
        "mask": np.triu(np.ones((128, 128), BF16NP)),
    }
    idx = f["idx"].astype(BF16NP)
    mem = np.zeros((B, TP, EP), BF16NP)
    mem[:, :T, :E] = f["memory"].astype(BF16NP)
    in_maps = []
    for c in range(NCORES):
        m = dict(shared)
        m["idx"] = np.ascontiguousarray(idx[c * BL:(c + 1) * BL])
        m["mem"] = np.ascontiguousarray(mem[c * BL:(c + 1) * BL])
        in_maps.append(m)
    return in_maps


_NC_CACHE = {}


def kernel(**inputs):
    if BL not in _NC_CACHE:
        _NC_CACHE[BL] = build_nc(BL)
    nc = _NC_CACHE[BL]
    in_maps = prepare_inputs(inputs)
    res = run_bass_kernel_spmd(nc, in_maps, list(range(NCORES)))
    return np.concatenate([res.results[c]["out"] for c in range(NCORES)], axis=0)


# revision 26
# speedup vs baseline: 1.8507x; 1.8507x over previous
"""Trainium2 Bass kernel for nn_Decoder (dense transformer decoder layer).

Strategy: pure data-parallel over batch B=256 across 8 NeuronCores (32
batches/core).  Each core runs the full decoder layer on its shard; no
collectives.

v2 design (vs baseline):
  - residual stream bf16 token-major; idx/mem shipped bf16 (halves input DMA)
  - LayerNorm stats via DVE bn_stats/bn_aggr; rsqrt = Exp(-0.5*Ln(var+eps))
    so the ONLY ACT table set used kernel-wide is natural_log_exp_and_others
    (exp/ln/identity/copy/relu) -> no table switching
  - memory's feature-major transpose done by DMA-transpose engines (bf16)
  - softmax denominator fused into the AV matmul via a per-head ones column
    appended to V (psum row 73 = Z); 1/Z via reciprocal_approx_fast
  - FFN bias folded into w1 as an extra K row against a ones-row in h3f;
    ReLU fused at the PSUM evict on ACT
  - per-head q/k slots evicted two heads at a time (one PSUM bank each)
  - samples processed two-at-a-time with stage-interleaved program order so
    the PE never sits behind a LayerNorm (HAM stays warm)
"""

import sys

sys.path.insert(0, "/opt/trn_rl_repo")

from contextlib import ExitStack

import numpy as np
import ml_dtypes

import concourse.bass as bass
import concourse.bacc as bacc
import concourse.mybir as mybir
import concourse.tile as tile
from concourse.bass_utils import run_bass_kernel_spmd

F32 = mybir.dt.float32
BF16 = mybir.dt.bfloat16
BF16NP = ml_dtypes.bfloat16
AF = mybir.ActivationFunctionType

B, T, E, H = 256, 200, 584, 8
HS = E // H  # 73
FF = 4 * E  # 2336
NCORES = 8
BL = B // NCORES  # 32
SCALE = float(E) ** -0.5
EPS = 1e-5
TP = 208  # mem DRAM padded token dim (multiple of 16 for DMA transpose)
EP = 640  # mem DRAM padded feature dim (multiple of 128)

# tile decompositions
TS = [(0, 128), (1, 72)]  # token tiles (T=200)
EB = [128, 128, 128, 128, 72]  # E=584 partition blocks
EK = 5
FFB = [128] * 18 + [32]  # FF=2336 partition blocks
FFK = 19
NSPL = [0, 292]  # free-dim split of an E-sized matmul output (<=512 psum)
HSZ = HS  # v slot width per head


def build_nc(bl=BL):
    """Build the single-core Bass program processing `bl` batch samples."""
    nc = bacc.Bacc(None, target_bir_lowering=False, debug=False)

    idx_d = nc.dram_tensor("idx", [bl, T, E], BF16, kind="ExternalInput")
    mem_d = nc.dram_tensor("mem", [bl, TP, EP], BF16, kind="ExternalInput")
    w_names = ["wq_sa", "wk_sa", "wv_sa", "wq_ca", "wk_ca", "wv_ca"]
    w_d = {n: nc.dram_tensor(n, [128, EK, E], BF16, kind="ExternalInput") for n in w_names}
    wp_sa_d = nc.dram_tensor("wp_sa", [128, H, E], BF16, kind="ExternalInput")
    wp_ca_d = nc.dram_tensor("wp_ca", [128, H, E], BF16, kind="ExternalInput")
    w1_d = nc.dram_tensor("w1", [128, EK, FF], BF16, kind="ExternalInput")
    w2_d = nc.dram_tensor("w2", [128, FFK, E], BF16, kind="ExternalInput")
    ident_d = nc.dram_tensor("ident", [128, 128], BF16, kind="ExternalInput")
    mask_d = nc.dram_tensor("mask", [128, 128], BF16, kind="ExternalInput")
    out_d = nc.dram_tensor("out", [bl, T, E], F32, kind="ExternalOutput")

    with tile.TileContext(nc) as tc, ExitStack() as ctx:
        wpool = ctx.enter_context(tc.tile_pool(name="wpool", bufs=1))
        w_sb = {}
        for n in w_names:
            w_sb[n] = wpool.tile([128, EK, E], BF16, name=n + "_sb")
            nc.sync.dma_start(w_sb[n][:], w_d[n][:])
        wp_sa_sb = wpool.tile([128, H, E], BF16, name="wp_sa_sb")
        nc.sync.dma_start(wp_sa_sb[:], wp_sa_d[:])
        wp_ca_sb = wpool.tile([128, H, E], BF16, name="wp_ca_sb")
        nc.sync.dma_start(wp_ca_sb[:], wp_ca_d[:])
        w1_sb = wpool.tile([128, EK, FF], BF16, name="w1_sb")
        nc.sync.dma_start(w1_sb[:], w1_d[:])
        w2_sb = wpool.tile([128, FFK, E], BF16, name="w2_sb")
        nc.sync.dma_start(w2_sb[:], w2_d[:])
        ident_sb = wpool.tile([128, 128], BF16, name="ident_sb")
        nc.sync.dma_start(ident_sb[:], ident_d[:])
        mask_sb = wpool.tile([128, 128], BF16, name="mask_sb")
        nc.sync.dma_start(mask_sb[:], mask_d[:])
        I32 = mybir.dt.int32
        shift1_sb = wpool.tile([128, 1], I32, name="shift1_sb")
        nc.vector.memset(shift1_sb[:], 1)
        ones_i_sb = wpool.tile([128, 1], I32, name="ones_i_sb")
        nc.vector.memset(ones_i_sb[:], -1)  # 0xFFFFFFFF for bitwise-not via xor
        magic_sb = wpool.tile([128, 2], I32, name="magic_sb")
        nc.vector.memset(magic_sb[:], 0x5F3759DF + 1)  # M+1 (M - x = ~x + M+1)
        ones_sb = wpool.tile([128, 1], BF16, name="ones_sb")
        nc.vector.memset(ones_sb[:], 1.0)

        resid = ctx.enter_context(tc.tile_pool(name="resid", bufs=2))
        work = ctx.enter_context(tc.tile_pool(name="work", bufs=2))
        stat = ctx.enter_context(tc.tile_pool(name="stat", bufs=4))
        opool = ctx.enter_context(tc.tile_pool(name="opool", bufs=2))
        ps_tp = ctx.enter_context(tc.tile_pool(name="ps_tp", bufs=1, space="PSUM"))
        ps_hp = ctx.enter_context(tc.tile_pool(name="ps_hp", bufs=2, space="PSUM"))
        ps_mm = ctx.enter_context(tc.tile_pool(name="ps_mm", bufs=2, space="PSUM"))
        ps_s = ctx.enter_context(tc.tile_pool(name="ps_s", bufs=2, space="PSUM"))
        ps_z = ctx.enter_context(tc.tile_pool(name="ps_z", bufs=1, space="PSUM"))

        def load(b):
            x1 = resid.tile([128, 2, E], BF16, name=f"x1_{b}", tag="x1", bufs=3)
            nc.sync.dma_start(x1[:, 0, :], idx_d[b, 0:128, :])
            nc.sync.dma_start(x1[0:72, 1, :], idx_d[b, 128:200, :])
            memf = work.tile([128, EK, TP], BF16, name=f"memf_{b}", tag="memf")
            for eb in range(EK):
                nc.sync.dma_start_transpose(
                    memf[:, eb, :], mem_d[b, :, eb * 128:eb * 128 + 128])
            return x1, memf

        def layernorm(x_t, name):
            """x_t [128,2,E] bf16 -> h_tok [128,2,E] bf16 normalized (no w/b).
            rsqrt via Newton iteration on DVE (quake seed + 2 NR passes) so no
            Sqrt/Ln activations are needed (single ACT table set kernel-wide)."""
            h_tok = work.tile([128, 2, E + 1], BF16, name=name, tag="htok", bufs=3)
            mv = stat.tile([128, 2, 2], F32, name=name + "_mv", tag="mv")
            for tt, tsz in TS:
                xs = x_t[0:tsz, tt, :]
                st = stat.tile([128, 2, 6], F32, name=name + f"_st_{tt}", tag="st")
                nc.vector.bn_stats(st[0:tsz, 0], xs[:, 0:292])
                nc.vector.bn_stats(st[0:tsz, 1], xs[:, 292:584])
                nc.vector.bn_aggr(mv[0:tsz, tt], st[0:tsz])
            AL = mybir.AluOpType
            vpe = stat.tile([128, 2], F32, name=name + "_vp", tag="vp")
            nc.vector.tensor_scalar(vpe[:], mv[:, :, 1], EPS, None, AL.add)
            r = stat.tile([128, 2], F32, name=name + "_r", tag="r")
            # seed bits = M - (v_bits >> 1) = ((v_bits >> 1) ^ ~0) + (M+1)
            nc.vector.tensor_scalar(
                r.bitcast(mybir.dt.int32)[:], vpe.bitcast(mybir.dt.int32)[:],
                shift1_sb[:], ones_i_sb[:], AL.arith_shift_right, AL.bitwise_xor)
            nc.vector.tensor_tensor(
                r.bitcast(mybir.dt.int32)[:], r.bitcast(mybir.dt.int32)[:],
                magic_sb[:], AL.add)
            for _ in range(1):  # r *= 1.5 - 0.5*v*r*r
                t = stat.tile([128, 2], F32, name=name + "_t", tag="t")
                nc.vector.tensor_mul(t[:], r[:], r[:])
                nc.vector.tensor_mul(t[:], t[:], vpe[:])
                nc.vector.tensor_scalar(t[:], t[:], -0.5, 1.5, AL.mult, AL.add)
                nc.vector.tensor_mul(r[:], r[:], t[:])
            nmr = stat.tile([128, 2], F32, name=name + "_nm", tag="nm")
            nc.vector.tensor_tensor(nmr[:], mv[:, :, 0], r[:], AL.mult)
            nc.vector.tensor_scalar(nmr[:], nmr[:], -1.0, None, AL.mult)
            for tt, tsz in TS:
                nc.scalar.activation(
                    h_tok[0:tsz, tt, 0:E], x_t[0:tsz, tt, :], AF.Identity,
                    bias=nmr[0:tsz, tt:tt + 1], scale=r[0:tsz, tt:tt + 1])
            return h_tok

        def to_fm(src_tok, name, ones_row=False):
            """[128,2,E(+1)] bf16 token-major -> [128,EK,T] bf16 feature-major.
            ones_row: src col E is set to 1.0 and carried through the block-4
            transpose into feature row 72 (the FFN bias row)."""
            dst = work.tile([128, EK, T], BF16, name=name, tag="hfm", bufs=3)
            e4 = 73 if ones_row else 72
            if ones_row:
                nc.vector.memset(src_tok[:, :, E:E + 1], 1.0)
            for tt, tsz in TS:
                ps = ps_tp.tile([128, EK, 128], BF16, name=f"{name}_tp{tt}", tag="tp")
                for eb in range(4):
                    nc.tensor.transpose(
                        ps[0:128, eb, 0:tsz],
                        src_tok[0:tsz, tt, eb * 128:eb * 128 + 128],
                        ident_sb[0:tsz, 0:tsz])
                nc.tensor.transpose(
                    ps[0:e4, 4, 0:tsz],
                    src_tok[0:tsz, tt, 512:512 + e4],
                    ident_sb[0:tsz, 0:tsz])
                nc.vector.tensor_copy(
                    dst[:, 0:4, tt * 128:tt * 128 + tsz], ps[:, 0:4, 0:tsz])
                nc.vector.tensor_copy(
                    dst[0:e4, 4, tt * 128:tt * 128 + tsz], ps[0:e4, 4, 0:tsz])
            return dst

        def proj_qk(w, src_fm, name):
            """q/k projection -> per-head aligned [HS, H, T] bf16."""
            dst = work.tile([HS, H, T], BF16, name=name, tag=name[:1])
            for hp in range(4):
                ps = ps_hp.tile([HS, 2, T], F32, name=f"{name}_ps{hp}", tag="hp")
                for h2 in range(2):
                    h = 2 * hp + h2
                    for k in range(EK):
                        nc.tensor.matmul(
                            ps[:, h2, :], w[0:EB[k], k, HS * h:HS * h + HS],
                            src_fm[0:EB[k], k, 0:T],
                            start=(k == 0), stop=(k == EK - 1))
                nc.scalar.copy(dst[:, 2 * hp:2 * hp + 2, :], ps[:])
            return dst

        def proj_v(w, src_fm, name):
            """v projection token-major with per-head ones column:
            [128, 2, H, HSZ] bf16 (col HS of each head slot = 1.0)."""
            dst = work.tile([128, 2, H, HSZ], BF16, name=name, tag="vtok")
            for mt, msz in TS:
                for ni, n0 in enumerate(NSPL):
                    ps = ps_mm.tile([128, 292], F32, name=f"{name}_ps{mt}{ni}", tag="mm")
                    for k in range(EK):
                        nc.tensor.matmul(
                            ps[0:msz, :],
                            src_fm[0:EB[k], k, mt * 128:mt * 128 + msz],
                            w[0:EB[k], k, n0:n0 + 292],
                            start=(k == 0), stop=(k == EK - 1))
                    nc.vector.tensor_copy(
                        dst[0:msz, mt, 4 * ni:4 * ni + 4, 0:HS],
                        ps[0:msz, :].rearrange("p (h d) -> p h d", h=4))
            return dst

        def attn_scores(q, k, causal, name):
            """scores + exp for all heads -> expS [128, 2, H, T] bf16.
            One exp per head covers both s-tiles (unwritten psum regions get
            exp'd into never-read expS slots; harmless)."""
            expS = opool.tile([128, 2, H, T], BF16, name=name, tag="expS")
            for h in range(H):
                ps = ps_s.tile([128, 2, T], F32, name=f"{name}_s{h}", tag="s")
                nc.tensor.matmul(
                    ps[0:128, 0, :], k[:, h, 0:128], q[:, h, :],
                    start=True, stop=True)
                t0 = 128 if causal else 0
                nc.tensor.matmul(
                    ps[0:72, 1, t0:T], k[:, h, 128:200], q[:, h, t0:T],
                    start=True, stop=True)
                nc.scalar.activation(
                    expS[0:128, 0, h, :], ps[0:128, 0, :], AF.Exp, scale=SCALE)
                nc.scalar.activation(
                    expS[0:72, 1, h, t0:T], ps[0:72, 1, t0:T], AF.Exp, scale=SCALE)
                if causal:
                    nc.vector.tensor_mul(
                        expS[0:128, 0, h, 0:128], expS[0:128, 0, h, 0:128],
                        mask_sb[0:128, 0:128])
                    nc.vector.tensor_mul(
                        expS[0:72, 1, h, 128:200], expS[0:72, 1, h, 128:200],
                        mask_sb[0:72, 0:72])
            return expS

        def attn_av(expS, v, causal, name):
            """AV with fused Z row -> normalized o [HS, H, T] bf16."""
            osb = opool.tile([HS, H, T], BF16, name=name, tag="osb")
            for hp in range(4):
                ps = ps_hp.tile([HSZ, 2, T], F32, name=f"{name}_o{hp}", tag="hp")
                for h2 in range(2):
                    h = 2 * hp + h2
                    if causal:
                        nc.tensor.matmul(
                            ps[:, h2, 0:128], v[0:128, 0, h, :],
                            expS[0:128, 0, h, 0:128], start=True, stop=True)
                        nc.tensor.matmul(
                            ps[:, h2, 128:200], v[0:128, 0, h, :],
                            expS[0:128, 0, h, 128:200], start=True, stop=False)
                        nc.tensor.matmul(
                            ps[:, h2, 128:200], v[0:72, 1, h, :],
                            expS[0:72, 1, h, 128:200], start=False, stop=True)
                    else:
                        nc.tensor.matmul(
                            ps[:, h2, :], v[0:128, 0, h, :],
                            expS[0:128, 0, h, :], start=True, stop=False)
                        nc.tensor.matmul(
                            ps[:, h2, :], v[0:72, 1, h, :],
                            expS[0:72, 1, h, :], start=False, stop=True)
                h0 = 2 * hp
                zps = ps_z.tile([1, 2, T], F32, name=f"{name}_z{hp}", tag="z")
                if causal:
                    nc.tensor.matmul(zps[0:1, :, :], ones_sb[0:128, 0:1],
                                     expS[0:128, 0, h0:h0 + 2, :],
                                     start=True, stop=False)
                    nc.tensor.matmul(zps[0:1, :, 128:200], ones_sb[0:72, 0:1],
                                     expS[0:72, 1, h0:h0 + 2, 128:200],
                                     start=False, stop=True)
                else:
                    nc.tensor.matmul(zps[0:1, :, :], ones_sb[0:128, 0:1],
                                     expS[0:128, 0, h0:h0 + 2, :],
                                     start=True, stop=False)
                    nc.tensor.matmul(zps[0:1, :, :], ones_sb[0:72, 0:1],
                                     expS[0:72, 1, h0:h0 + 2, :],
                                     start=False, stop=True)
                zr = stat.tile([1, 2, T], F32, name=f"{name}_zr{hp}", tag="zr")
                nc.vector.reciprocal_approx_fast(out=zr[0:1, :, :], in_=zps[0:1, :, :])
                for h2 in range(2):
                    h = h0 + h2
                    zb = stat.tile([HS, T], F32, name=f"{name}_zb{h}", tag="zb")
                    nc.gpsimd.partition_broadcast(zb[:, :], zr[0:1, h2, :])
                    nc.vector.tensor_mul(osb[:, h, :], ps[0:HS, h2, :], zb[:, :])
            return osb

        def attn_proj(osb, wp_sb, x_in, name, xtag, xbufs=2):
            """projection (accumulate over heads) + residual, token-major."""
            x_out = resid.tile([128, 2, E], BF16, name=f"{name}_xo", tag=xtag,
                               bufs=xbufs)
            for mt, msz in TS:
                for n0 in NSPL:
                    ps = ps_mm.tile([128, 292], F32, name=f"{name}_pj", tag="mm")
                    for h in range(H):
                        nc.tensor.matmul(
                            ps[0:msz, :],
                            osb[:, h, mt * 128:mt * 128 + msz],
                            wp_sb[0:HS, h, n0:n0 + 292],
                            start=(h == 0), stop=(h == H - 1))
                    nc.vector.tensor_add(
                        x_out[0:msz, mt, n0:n0 + 292], ps[0:msz, :],
                        x_in[0:msz, mt, n0:n0 + 292])
            return x_out

        def ffn_w1(h3f, b):
            ff = work.tile([128, FFK, T], BF16, name=f"ff_{b}", tag="ff")
            for m in range(FFK):
                msz = FFB[m]
                ps = ps_mm.tile([128, 292], F32, name=f"f1_{b}_{m}", tag="mm")
                for k in range(EK):
                    ksz = 73 if k == EK - 1 else EB[k]  # incl. bias ones-row
                    nc.tensor.matmul(
                        ps[0:msz, 0:T], w1_sb[0:ksz, k, m * 128:m * 128 + msz],
                        h3f[0:ksz, k, :], start=(k == 0), stop=(k == EK - 1))
                nc.vector.tensor_scalar(
                    ff[0:msz, m, :], ps[0:msz, 0:T], 0.0, None,
                    mybir.AluOpType.max)
            return ff

        def ffn_w2(ff, x3, b):
            xo = resid.tile([128, 2, E], F32, name=f"xo_{b}", tag="xo")
            for mt, msz in TS:
                for n0 in NSPL:
                    ps = ps_mm.tile([128, 292], F32, name=f"f2_{b}_{mt}_{n0}", tag="mm")
                    for k in range(FFK):
                        nc.tensor.matmul(
                            ps[0:msz, :], ff[0:FFB[k], k, mt * 128:mt * 128 + msz],
                            w2_sb[0:FFB[k], k, n0:n0 + 292],
                            start=(k == 0), stop=(k == FFK - 1))
                    nc.vector.tensor_add(
                        xo[0:msz, mt, n0:n0 + 292], ps[0:msz, :],
                        x3[0:msz, mt, n0:n0 + 292])
            return xo

        def store(xo, b):
            nc.sync.dma_start(out_d[b, 0:128, :], xo[:, 0, :])
            nc.sync.dma_start(out_d[b, 128:200, :], xo[0:72, 1, :])

        def stages(b):
            """Generator of per-sample stages; yields after each stage so two
            samples can be interleaved in program order."""
            x1, memf = load(b)
            yield
            h1 = layernorm(x1, f"ln1_{b}")
            yield
            h1f = to_fm(h1, f"h1f_{b}")
            yield
            q1 = proj_qk(w_sb["wq_sa"], h1f, f"q1_{b}")
            k1 = proj_qk(w_sb["wk_sa"], h1f, f"k1_{b}")
            v1 = proj_v(w_sb["wv_sa"], h1f, f"v1_{b}")
            yield
            e1 = attn_scores(q1, k1, True, f"se1_{b}")
            yield
            o1 = attn_av(e1, v1, True, f"av1_{b}")
            yield
            x2 = attn_proj(o1, wp_sa_sb, x1, f"sa_{b}", "x2")
            yield
            h2 = layernorm(x2, f"ln2_{b}")
            yield
            h2f = to_fm(h2, f"h2f_{b}")
            yield
            q2 = proj_qk(w_sb["wq_ca"], h2f, f"q2_{b}")
            k2 = proj_qk(w_sb["wk_ca"], memf, f"k2_{b}")
            v2 = proj_v(w_sb["wv_ca"], h2f, f"v2_{b}")
            yield
            e2 = attn_scores(q2, k2, False, f"se2_{b}")
            yield
            o2 = attn_av(e2, v2, False, f"av2_{b}")
            yield
            x3 = attn_proj(o2, wp_ca_sb, x2, f"ca_{b}", "x3")
            yield
            h3 = layernorm(x3, f"ln3_{b}")
            yield
            h3f = to_fm(h3, f"h3f_{b}", ones_row=True)
            yield
            ff = ffn_w1(h3f, b)
            yield
            xo = ffn_w2(ff, x3, b)
            yield
            store(xo, b)

        # software-pipeline the samples: each sample starts HALF stages after
        # the previous one, so LayerNorm/softmax phases of one sample overlap
        # the matmul phases of its neighbor and the PE never idles (HAM warm).
        HALF = 9
        active = []
        t = 0
        next_s = 0
        while next_s < bl or active:
            if next_s < bl and t >= next_s * HALF:
                active.append(stages(next_s))
                next_s += 1
            for g in list(active):
                if next(g, "end") == "end":
                    active.remove(g)
            t += 1

    nc.compile()
    return nc


def _pack_kxm(w, nk, extra_row=None):
    """[K, M] fp32 -> [128, nk, M] bf16 with K zero-padded to 128*nk.
    extra_row, if given, is placed at global row K (the first pad row)."""
    K, M = w.shape
    pad = np.zeros((128 * nk, M), np.float32)
    pad[:K] = w
    if extra_row is not None:
        pad[K] = extra_row
    return np.ascontiguousarray(
        pad.reshape(nk, 128, M).transpose(1, 0, 2)).astype(BF16NP)


def prepare_inputs(inputs):
    """Host-side prep: LN folding, weight packing, per-core sharding."""
    f = {k: np.asarray(v, np.float32) for k, v in inputs.items()}

    def fold(lnw, lnb, w3):
        wf = w3 * lnw[None, :, None]
        bias = np.einsum("e,hed->hd", lnb, w3) if lnb.any() else 0.0
        assert np.allclose(bias, 0.0, atol=1e-12), "nonzero folded qkv bias unsupported"
        return wf

    sa_q = fold(f["ln1_w"], f["ln1_b"], f["sa_q"])
    sa_k = fold(f["ln1_w"], f["ln1_b"], f["sa_k"])
    sa_v = fold(f["ln1_w"], f["ln1_b"], f["sa_v"])
    ca_q = fold(f["ln2_w"], f["ln2_b"], f["ca_q"])
    ca_v = fold(f["ln2_w"], f["ln2_b"], f["ca_v"])
    ca_k = f["ca_k"]  # cross-attn keys come from raw memory (no LN)
    w1 = f["ff_w1"] * f["ln3_w"][:, None]
    b1 = f["ff_b1"] + f["ln3_b"] @ f["ff_w1"]
    assert np.allclose(f["sa_pb"], 0.0) and np.allclose(f["ca_pb"], 0.0), \
        "nonzero attn proj bias unsupported"
    assert np.allclose(f["ff_b2"], 0.0), "nonzero ff_b2 unsupported"

    def stack_heads(w3):  # [H, E, HS] -> [E, H*HS]
        return np.ascontiguousarray(w3.transpose(1, 0, 2)).reshape(E, E)

    def pack_proj(pw):  # [E, E] -> [128(73 used), H, E] per-head K layout
        r = pw.reshape(H, HS, E)
        out = np.zeros((H, 128, E), np.float32)
        out[:, :HS, :] = r
        return np.ascontiguousarray(out.transpose(1, 0, 2)).astype(BF16NP)

    shared = {
        "wq_sa": _pack_kxm(stack_heads(sa_q), EK),
        "wk_sa": _pack_kxm(stack_heads(sa_k), EK),
        "wv_sa": _pack_kxm(stack_heads(sa_v), EK),
        "wq_ca": _pack_kxm(stack_heads(ca_q), EK),
        "wk_ca": _pack_kxm(stack_heads(ca_k), EK),
        "wv_ca": _pack_kxm(stack_heads(ca_v), EK),
        "wp_sa": pack_proj(f["sa_pw"]),
        "wp_ca": pack_proj(f["ca_pw"]),
        "w1": _pack_kxm(w1, EK, extra_row=b1),
        "w2": _pack_kxm(f["ff_w2"], FFK),
        "ident": np.eye(128, dtype=BF16NP),
        "mask": np.triu(np.ones((128, 128), BF16NP)),
    }
    idx = f["idx"].astype(BF16NP)
    mem = np.zeros((B, TP, EP), BF16NP)
    mem[:, :T, :E] = f["memory"].astype(BF16NP)
    in_maps = []
    for c in range(NCORES):
        m = dict(shared)
        m["idx"] = np.ascontiguousarray(idx[c * BL:(c + 1) * BL])
        m["mem"] = np.ascontiguousarray(mem[c * BL:(c + 1) * BL])
        in_maps.append(m)
    return in_maps


_NC_CACHE = {}


def kernel(**inputs):
    if BL not in _NC_CACHE:
        _NC_CACHE[BL] = build_nc(BL)
    nc = _NC_CACHE[BL]
    in_maps = prepare_inputs(inputs)
    res = run_bass_kernel_spmd(nc, in_maps, list(range(NCORES)))
    return np.concatenate([res.results[c]["out"] for c in range(NCORES)], axis=0)
